# revision 26
# baseline (speedup 1.0000x reference)
"""BondInfluenceSelfAttention TRN2 kernel (v2).

Full-input contract: kernel(**inputs) takes the complete unsharded inputs and
returns the full [B, L, D] output. Internally shards across 8 NeuronCores:
core c handles batch b = c // 4 and head-group g = c % 4 (4 heads, 256 dk
dims). Each core computes its heads' attention plus the partial output
projection through its 256 rows of Wo; the host sums the 4 partials per batch
and adds bo.

v2 design (vs the v1 in git history):
- bf16 operands for every matmul (proj, scores, PV, out-proj); fp32 PSUM.
- Score matmuls (K=64) packed two-per-array via tile_position row groups;
  PV matmuls (M=64) packed two-per-array via column groups.
- Softmax denominators from 4-way column-packed ones^T @ P matmuls into one
  PSUM bank; normalization deferred to the chunk tail (reciprocal_approx_fast
  on DVE + K=1 broadcast matmuls), killing v1's 53us of DVE RECIPROCAL.
- ACT does only: exp in [128,4096] ops, Q/K/V PSUM->SBUF copies (with bias),
  Y copies. DVE does only: bond-multiply (PSUM->SBUF), recip, normalize.
- QKV projection matmul groups are interleaved into chunk 0's attention steps
  (just-in-time spans) so DVE/ACT never sit idle behind a serial proj phase.
"""

import numpy as np

try:
    import concourse.bass as bass  # noqa: F401
except ImportError:  # pragma: no cover
    import sys

    sys.path.insert(0, "/opt/trn_rl_repo")
    import concourse.bass as bass  # noqa: F401

import concourse.bacc as bacc
import concourse.mybir as mybir
import concourse.tile as tile
from concourse.bass_utils import run_bass_kernel_spmd
from ml_dtypes import bfloat16

F32 = mybir.dt.float32
F32R = mybir.dt.float32r
BF16 = mybir.dt.bfloat16
F16 = mybir.dt.float16

D = 1024  # d_model
L = 2048  # sequence length
B = 2  # batch
HPC = 4  # heads per core
DKG = 256  # dk dims per core (4 heads x 64)
NK = D // 128  # 8 contraction k-tiles for the projections
LT = L // 128  # 16 L k-position tiles
NCH = L // 512  # 4 L_q chunks
NSP = 4  # 512-wide k-position spans (kt/vt production granularity)
N_CORES = 8

_CACHED_NC = None


def _build_nc():
    nc = bacc.Bacc("TRN2", target_bir_lowering=False, debug=False, num_devices=N_CORES)

    xt_d = nc.declare_dram_parameter("xt", [D, L], BF16, isOutput=False)
    bd_d = nc.declare_dram_parameter("bd", [L, L], F16, isOutput=False)
    wq_d = nc.declare_dram_parameter("wq", [D, DKG], BF16, isOutput=False)
    wk_d = nc.declare_dram_parameter("wk", [D, DKG], BF16, isOutput=False)
    wv_d = nc.declare_dram_parameter("wv", [D, DKG], BF16, isOutput=False)
    bqk_d = nc.declare_dram_parameter("bqk", [128, 4], F32, isOutput=False)
    bv_d = nc.declare_dram_parameter("bv", [1, DKG], BF16, isOutput=False)
    wo_d = nc.declare_dram_parameter("wo", [DKG, D], BF16, isOutput=False)
    y_d = nc.declare_dram_parameter("y", [L, D], BF16, isOutput=True)

    Exp = mybir.ActivationFunctionType.Exp
    Identity = mybir.ActivationFunctionType.Identity

    with tile.TileContext(nc) as tc:
        with tc.tile_pool(name="persist", bufs=1) as pp, tc.tile_pool(
            name="work", bufs=1
        ) as wk_pool, tc.tile_pool(name="ps", bufs=1, space="PSUM") as ps:
            # ---- persistent SBUF ----
            xk = [
                pp.tile([128, L], BF16, tag=f"xk{k}", name=f"xk{k}")
                for k in range(NK)
            ]
            wq_sb = pp.tile([128, NK, DKG], BF16, tag="wq", name="wq_sb")
            wk_sb = pp.tile([128, NK, DKG], BF16, tag="wk", name="wk_sb")
            wv_sb = pp.tile([128, NK, DKG], BF16, tag="wv", name="wv_sb")
            qt = [pp.tile([128, L], BF16, tag=f"qt{t}", name=f"qt{t}") for t in range(2)]
            kt = [pp.tile([128, L], BF16, tag=f"kt{t}", name=f"kt{t}") for t in range(2)]
            vt = pp.tile([128, LT, DKG], BF16, tag="vt", name="vt")
            st = [
                pp.tile([128, 4, 1024], F32, tag=f"st{g}", name=f"st{g}")
                for g in range(2)
            ]
            pt = [
                pp.tile([128, 4, 1024], BF16, tag=f"pt{g}", name=f"pt{g}")
                for g in range(2)
            ]
            wo_sb = pp.tile([128, 2, D], BF16, tag="wo", name="wo_sb")
            bqk_sb = pp.tile([128, 4], F32, tag="bqk", name="bqk_sb")
            bv_sb = pp.tile([1, DKG], BF16, tag="bv", name="bv_sb")
            ones_r = pp.tile([1, 128], BF16, tag="onesr", name="ones_r")
            onesb = pp.tile([128, 128], BF16, tag="onesb", name="onesb")
            ones_f = pp.tile([128, 128], F32, tag="onesf", name="ones_f")
            # sel[:, t, :]: K=128 selection matrix broadcasting denominator
            # rows {64t, 64t+32} of dsb to output partitions [0:64], [64:128]
            sel = pp.tile([128, 2, 128], F32R, tag="sel", name="sel")
            sel_f = pp.tile([128, 2, 128], F32, tag="self", name="sel_f")

            # ---- input DMA: weights first (gate first proj groups), x spans
            # batched one-per-span and spread across engine queues ----
            # full x rows per k-tile: each partition line is one contiguous
            # 4KB DRAM row, spread across the three DMA-capable queues
            nc.sync.dma_start(out=wk_sb, in_=wk_d.ap().rearrange("(k p) n -> p k n", p=128))
            nc.scalar.dma_start(out=wq_sb, in_=wq_d.ap().rearrange("(k p) n -> p k n", p=128))
            nc.gpsimd.dma_start(out=wv_sb, in_=wv_d.ap().rearrange("(k p) n -> p k n", p=128))
            nc.scalar.dma_start(out=bqk_sb, in_=bqk_d[:, :])
            nc.scalar.dma_start(out=bv_sb, in_=bv_d[:, :])
            qs = [nc.sync, nc.scalar, nc.gpsimd]
            for k in range(NK):
                qs[k % 3].dma_start(out=xk[k], in_=xt_d[128 * k : 128 * (k + 1), :])
            nc.gpsimd.dma_start(out=wo_sb, in_=wo_d.ap().rearrange("(t p) n -> p t n", p=128))
            nc.vector.memset(ones_f, 1.0)
            nc.vector.tensor_copy(out=onesb, in_=ones_f)
            nc.vector.tensor_copy(out=ones_r, in_=ones_f[0:1, :])
            nc.vector.memset(sel_f, 0.0)
            for t in range(2):
                nc.vector.memset(sel_f[64 * t : 64 * t + 1, t, 0:64], 1.0)
                nc.vector.memset(sel_f[64 * t + 32 : 64 * t + 33, t, 64:128], 1.0)
            nc.vector.tensor_copy(out=sel, in_=sel_f)

            # ---- projection group emitters ----
            def kt_group(t, s):
                pb = ps.tile([128, 512], F32, tag="pj", name="pj")
                for k in range(NK):
                    nc.tensor.matmul(
                        pb[:, :],
                        wk_sb[:, k, 128 * t : 128 * (t + 1)],
                        xk[k][:, 512 * s : 512 * (s + 1)],
                        start=(k == 0),
                        stop=(k == NK - 1),
                    )
                nc.scalar.activation(
                    out=kt[t][:, 512 * s : 512 * (s + 1)],
                    in_=pb[:, :],
                    func=Identity,
                    bias=bqk_sb[:, 2 + t : 3 + t],
                )

            def qt_group(t, c):
                pb = ps.tile([128, 512], F32, tag="pj", name="pj")
                for k in range(NK):
                    nc.tensor.matmul(
                        pb[:, :],
                        wq_sb[:, k, 128 * t : 128 * (t + 1)],
                        xk[k][:, 512 * c : 512 * (c + 1)],
                        start=(k == 0),
                        stop=(k == NK - 1),
                    )
                nc.scalar.activation(
                    out=qt[t][:, 512 * c : 512 * (c + 1)],
                    in_=pb[:, :],
                    func=Identity,
                    bias=bqk_sb[:, t : t + 1],
                )

            def vt_group(ii):  # i-tiles 2*ii, 2*ii+1
                pb = ps.tile([128, 512], F32, tag="pj", name="pj")
                for j in range(2):
                    i = 2 * ii + j
                    for k in range(NK):
                        nc.tensor.matmul(
                            pb[:, 256 * j : 256 * (j + 1)],
                            xk[k][:, 128 * i : 128 * (i + 1)],
                            wv_sb[:, k, :],
                            start=(k == 0),
                            stop=False,
                        )
                    nc.tensor.matmul(
                        pb[:, 256 * j : 256 * (j + 1)],
                        ones_r,
                        bv_sb,
                        start=False,
                        stop=True,
                    )
                nc.scalar.activation(
                    out=vt[:, 2 * ii : 2 * ii + 2, :],
                    in_=pb.rearrange("p (j n) -> p j n", j=2),
                    func=Identity,
                )

            # fine-grained qt emitter: 2 matmuls per slot, psum bank held
            # across the slots of one group to avoid 8-MM pacer stalls
            qt_pb = {}

            def qt_part(t, c, k0, k1):
                key = (t, c)
                if key not in qt_pb:
                    qt_pb[key] = ps.tile([128, 512], F32, tag="pj", name="pj")
                pb = qt_pb[key]
                for k in range(k0, k1):
                    nc.tensor.matmul(
                        pb[:, :],
                        wq_sb[:, k, 128 * t : 128 * (t + 1)],
                        xk[k][:, 512 * c : 512 * (c + 1)],
                        start=(k == 0),
                        stop=(k == NK - 1),
                    )
                if k1 == NK:
                    nc.scalar.activation(
                        out=qt[t][:, 512 * c : 512 * (c + 1)],
                        in_=pb[:, :],
                        func=Identity,
                        bias=bqk_sb[:, t : t + 1],
                    )
                    del qt_pb[key]

            # just-in-time schedule: chunk-0 steps produce the remaining spans
            slots = {
                (0, 0): [lambda: vt_group(0)],
                (0, 1): [lambda: vt_group(1)],
                (0, 2): [lambda: kt_group(0, 1)],
                (0, 3): [lambda: kt_group(1, 1)],
                (0, 4): [lambda: vt_group(2)],
                (0, 5): [lambda: vt_group(3)],
                (0, 6): [lambda: kt_group(0, 2)],
                (0, 7): [lambda: kt_group(1, 2)],
                (0, 8): [lambda: vt_group(4)],
                (0, 9): [lambda: vt_group(5)],
                (0, 10): [lambda: kt_group(0, 3)],
                (0, 11): [lambda: kt_group(1, 3)],
                (0, 12): [lambda: vt_group(6)],
                (0, 13): [lambda: vt_group(7)],
                (0, 14): [lambda: qt_group(0, 1)],
                (0, 15): [lambda: qt_group(1, 1)],
            }
            for c in (1, 2):
                cn = c + 1
                for p in range(4):
                    slots[(c, 2 + p)] = [
                        lambda t=0, cn=cn, p=p: qt_part(t, cn, 2 * p, 2 * p + 2)
                    ]
                    slots[(c, 6 + p)] = [
                        lambda t=1, cn=cn, p=p: qt_part(t, cn, 2 * p, 2 * p + 2)
                    ]

            # bootstrap (overlapped with the span-0 DMA): kt span 0 + qt c0
            kt_group(0, 0)
            kt_group(1, 0)
            qt_group(0, 0)
            qt_group(1, 0)

            # ---- bond DMA ring ----
            steps = [(c, i) for c in range(NCH) for i in range(LT)]
            bts = {}

            def bond_dma(n):
                if n >= len(steps):
                    return
                c, i = steps[n]
                bt = wk_pool.tile([128, 512], F16, tag="bt", bufs=6, name="bt")
                nc.sync.dma_start(
                    out=bt,
                    in_=bd_d[128 * i : 128 * (i + 1), 512 * c : 512 * (c + 1)],
                )
                bts[n] = bt

            for n in range(4):
                bond_dma(n)

            # ---- attention ----
            def pv_step(c, j, oacc, dn):
                first, last = (j == 0), (j == LT - 1)
                g = (j // 2) % 2
                for t in range(2):
                    idx = (j % 2) * 2 + t
                    for half in range(2):
                        h = 2 * t + half
                        nc.tensor.matmul(
                            oacc[t][64 * half : 64 * (half + 1), :],
                            vt[:, j, 64 * h : 64 * (h + 1)],
                            pt[g][:, idx, 512 * half : 512 * (half + 1)],
                            start=first,
                            stop=last,
                            tile_position=(0, 64 * half),
                        )
                for h in range(HPC):
                    t, half = h // 2, h % 2
                    idx = (j % 2) * 2 + t
                    nc.tensor.matmul(
                        dn[32 * h : 32 * h + 1, :],
                        onesb[:, 0:1],
                        pt[g][:, idx, 512 * half : 512 * (half + 1)],
                        start=first,
                        stop=last,
                        tile_position=(0, 32 * h),
                    )

            for c in range(NCH):
                oacc = [
                    ps.tile([128, 512], F32, tag=f"o{t}", name=f"oacc{t}")
                    for t in range(2)
                ]
                dn = ps.tile([128, 512], F32, tag="dn", name="dn")
                for i in range(LT):
                    n = c * LT + i
                    # keep-warm: standalone background LDWEIGHTS so HAM never
                    # re-throttles the PE clock during short dependency stalls
                    nc.tensor.ldweights(onesb[:, :])
                    for fn in slots.get((c, i), ()):
                        fn()
                    bond_dma(n + 4)
                    bt = bts.pop(n)
                    bt_b = bass.AP(
                        tensor=bt.tensor,
                        offset=bt.offset,
                        ap=[bt.ap[0], [0, 2]] + list(bt.ap[1:]),
                    )
                    g = (i // 2) % 2
                    for t in range(2):
                        sp = ps.tile([128, 2, 512], F32, tag="s", bufs=2, name="sp")
                        nc.tensor.matmul(
                            sp[:, 0, :],
                            kt[t][0:64, 128 * i : 128 * (i + 1)],
                            qt[t][0:64, 512 * c : 512 * (c + 1)],
                            start=True,
                            stop=True,
                        )
                        nc.tensor.matmul(
                            sp[:, 1, :],
                            kt[t][64:128, 128 * i : 128 * (i + 1)],
                            qt[t][64:128, 512 * c : 512 * (c + 1)],
                            start=True,
                            stop=True,
                            tile_position=(64, 0),
                        )
                        idx = (i % 2) * 2 + t
                        out_view = st[g][:, idx, :].rearrange("p (h q) -> p h q", h=2)
                        nc.vector.tensor_mul(out=out_view, in0=sp, in1=bt_b)
                    if i % 2 == 1:
                        with nc.allow_low_precision(reason="bf16 probs"):
                            nc.scalar.activation(out=pt[g], in_=st[g], func=Exp)
                    if i >= 2:
                        pv_step(c, i - 2, oacc, dn)
                pv_step(c, LT - 2, oacc, dn)
                pv_step(c, LT - 1, oacc, dn)

                # ---- chunk tail: denominators, normalize, out-proj ----
                dsb = wk_pool.tile([128, 512], F32R, tag="dsb", bufs=2, name="dsb")
                nc.scalar.activation(out=dsb, in_=dn, func=Identity)
                bcb = [
                    ps.tile([128, 512], F32, tag=("dn" if t == 0 else "pj"), name="bcb")
                    for t in range(2)
                ]
                for t in range(2):
                    nc.tensor.matmul(
                        bcb[t][:, :],
                        sel[:, t, :],
                        dsb[:, :],
                        start=True,
                        stop=True,
                    )
                rb = [
                    wk_pool.tile([128, 512], F32, tag="rb", bufs=2, name="rb")
                    for _ in range(2)
                ]
                on = wk_pool.tile([128, 2, 512], BF16, tag="on", bufs=2, name="on")
                with nc.allow_low_precision(reason="bf16 normalized O"):
                    for t in range(2):
                        nc.vector.reciprocal_approx_fast(out=rb[t], in_=bcb[t])
                        nc.vector.tensor_mul(out=on[:, t, :], in0=oacc[t], in1=rb[t])
                for jl in range(4):
                    j = 4 * c + jl
                    for dh in range(2):
                        yp = ps.tile(
                            [128, 512],
                            F32,
                            tag=("pj" if (2 * jl + dh) % 2 else "dn"),
                            name="yp",
                        )
                        for t in range(2):
                            nc.tensor.matmul(
                                yp[:, :],
                                on[:, t, 128 * jl : 128 * (jl + 1)],
                                wo_sb[:, t, 512 * dh : 512 * (dh + 1)],
                                start=(t == 0),
                                stop=(t == 1),
                            )
                        ys = wk_pool.tile([128, 512], BF16, tag="ys", bufs=4, name="ys")
                        with nc.allow_low_precision(reason="bf16 partial Y"):
                            nc.scalar.activation(out=ys, in_=yp, func=Identity)
                        nc.gpsimd.dma_start(
                            out=y_d[128 * j : 128 * (j + 1), 512 * dh : 512 * (dh + 1)],
                            in_=ys,
                        )

    nc.compile()
    return nc


def _get_nc():
    global _CACHED_NC
    if _CACHED_NC is None:
        _CACHED_NC = _build_nc()
    return _CACHED_NC


def _host_prep(x, bond_influence, Wq, bq, Wk, bk, Wv, bv, Wo):
    in_maps = []
    for core in range(N_CORES):
        b, g = core // HPC, core % HPC
        s = slice(g * DKG, (g + 1) * DKG)
        bq_g = (bq[s] / 8.0).astype(np.float32)
        bk_g = bk[s].astype(np.float32)
        bqk = np.stack(
            [bq_g[0:128], bq_g[128:256], bk_g[0:128], bk_g[128:256]], axis=1
        )
        in_maps.append(
            {
                "xt": np.ascontiguousarray(x[b].T).astype(bfloat16),
                "bd": np.ascontiguousarray(bond_influence[b].T.astype(np.float16)),
                "wq": np.ascontiguousarray(Wq[:, s] / 8.0).astype(bfloat16),
                "wk": np.ascontiguousarray(Wk[:, s]).astype(bfloat16),
                "wv": np.ascontiguousarray(Wv[:, s]).astype(bfloat16),
                "bqk": np.ascontiguousarray(bqk),
                "bv": np.ascontiguousarray(bv[s][None, :]).astype(bfloat16),
                "wo": np.ascontiguousarray(Wo[s, :]).astype(bfloat16),
            }
        )
    return in_maps


def kernel(
    x,
    bond_influence,
    Wq,
    bq,
    Wk,
    bk,
    Wv,
    bv,
    Wo,
    bo,
    _trace=False,
    _trace_out=None,
):
    x = np.asarray(x, dtype=np.float32)
    bond_influence = np.asarray(bond_influence, dtype=np.float32)
    args = [np.asarray(a, dtype=np.float32) for a in (Wq, bq, Wk, bk, Wv, bv, Wo)]
    bo = np.asarray(bo, dtype=np.float32)

    nc = _get_nc()
    in_maps = _host_prep(x, bond_influence, *args)
    kwargs = {}
    if _trace:
        kwargs = dict(trace=True, tmpdir=_trace_out)
    res = run_bass_kernel_spmd(nc, in_maps, list(range(N_CORES)), **kwargs)

    out = np.zeros((B, L, D), dtype=np.float32)
    for b in range(B):
        acc = res.results[4 * b]["y"].astype(np.float32)
        for g in range(1, HPC):
            acc = acc + res.results[4 * b + g]["y"].astype(np.float32)
        out[b] = acc + bo[None, :]
    if _trace:
        return out, res
    return out


# revision 32
# speedup vs baseline: 1.3143x; 1.3143x over previous
"""BondInfluenceSelfAttention TRN2 kernel (v2).

Full-input contract: kernel(**inputs) takes the complete unsharded inputs and
returns the full [B, L, D] output. Internally shards across 8 NeuronCores:
core c handles batch b = c // 4 and head-group g = c % 4 (4 heads, 256 dk
dims). Each core computes its heads' attention plus the partial output
projection through its 256 rows of Wo; the host sums the 4 partials per batch
and adds bo.

v2 design (vs the v1 in git history):
- bf16 operands for every matmul (proj, scores, PV, out-proj); fp32 PSUM.
- Score matmuls (K=64) packed two-per-array via tile_position row groups;
  PV matmuls (M=64) packed two-per-array via column groups.
- Softmax denominators from 4-way column-packed ones^T @ P matmuls into one
  PSUM bank; normalization deferred to the chunk tail (reciprocal_approx_fast
  on DVE + K=1 broadcast matmuls), killing v1's 53us of DVE RECIPROCAL.
- ACT does only: exp in [128,4096] ops, Q/K/V PSUM->SBUF copies (with bias),
  Y copies. DVE does only: bond-multiply (PSUM->SBUF), recip, normalize.
- QKV projection matmul groups are interleaved into chunk 0's attention steps
  (just-in-time spans) so DVE/ACT never sit idle behind a serial proj phase.
"""

import numpy as np

try:
    import concourse.bass as bass  # noqa: F401
except ImportError:  # pragma: no cover
    import sys

    sys.path.insert(0, "/opt/trn_rl_repo")
    import concourse.bass as bass  # noqa: F401

import concourse.bacc as bacc
import concourse.mybir as mybir
import concourse.tile as tile
from concourse.bass_utils import run_bass_kernel_spmd
from ml_dtypes import bfloat16

F32 = mybir.dt.float32
F32R = mybir.dt.float32r
BF16 = mybir.dt.bfloat16
F16 = mybir.dt.float16

D = 1024  # d_model
L = 2048  # sequence length
B = 2  # batch
HPC = 4  # heads per core
DKG = 256  # dk dims per core (4 heads x 64)
NK = D // 128  # 8 contraction k-tiles for the projections
LT = L // 128  # 16 L k-position tiles
NCH = L // 512  # 4 L_q chunks
NSP = 4  # 512-wide k-position spans (kt/vt production granularity)
N_CORES = 8

_CACHED_NC = None


def _build_nc():
    nc = bacc.Bacc("TRN2", target_bir_lowering=False, debug=False, num_devices=N_CORES)

    xt_d = nc.declare_dram_parameter("xt", [D, L], BF16, isOutput=False)
    bd_d = nc.declare_dram_parameter("bd", [L, L], F16, isOutput=False)
    wq_d = nc.declare_dram_parameter("wq", [D, DKG], BF16, isOutput=False)
    wk_d = nc.declare_dram_parameter("wk", [D, DKG], BF16, isOutput=False)
    wv_d = nc.declare_dram_parameter("wv", [D, DKG], BF16, isOutput=False)
    bqk_d = nc.declare_dram_parameter("bqk", [128, 4], F32, isOutput=False)
    bv_d = nc.declare_dram_parameter("bv", [1, DKG], BF16, isOutput=False)
    wo_d = nc.declare_dram_parameter("wo", [DKG, D], BF16, isOutput=False)
    y_d = nc.declare_dram_parameter("y", [L, D], BF16, isOutput=True)

    Exp = mybir.ActivationFunctionType.Exp
    Identity = mybir.ActivationFunctionType.Identity

    with tile.TileContext(nc) as tc:
        with tc.tile_pool(name="persist", bufs=1) as pp, tc.tile_pool(
            name="work", bufs=1
        ) as wk_pool, tc.tile_pool(name="ps", bufs=1, space="PSUM") as ps:
            # ---- persistent SBUF ----
            xk = [
                pp.tile([128, L], BF16, tag=f"xk{k}", name=f"xk{k}")
                for k in range(NK)
            ]
            wq_sb = pp.tile([128, NK, DKG], BF16, tag="wq", name="wq_sb")
            wk_sb = pp.tile([128, NK, DKG], BF16, tag="wk", name="wk_sb")
            wv_sb = pp.tile([128, NK, DKG], BF16, tag="wv", name="wv_sb")
            qt = [pp.tile([128, L], BF16, tag=f"qt{t}", name=f"qt{t}") for t in range(2)]
            kt = [pp.tile([128, L], BF16, tag=f"kt{t}", name=f"kt{t}") for t in range(2)]
            vt = pp.tile([128, LT, DKG], BF16, tag="vt", name="vt")
            st = [
                pp.tile([128, 4, 1024], F32, tag=f"st{g}", name=f"st{g}")
                for g in range(3)
            ]
            pt = [
                pp.tile([128, 4, 1024], BF16, tag=f"pt{g}", name=f"pt{g}")
                for g in range(3)
            ]
            wo_sb = pp.tile([128, 2, D], BF16, tag="wo", name="wo_sb")
            bqk_sb = pp.tile([128, 4], F32, tag="bqk", name="bqk_sb")
            bv_sb = pp.tile([1, DKG], BF16, tag="bv", name="bv_sb")
            ones_r = pp.tile([1, 128], BF16, tag="onesr", name="ones_r")
            onesb = pp.tile([128, 128], BF16, tag="onesb", name="onesb")
            ones_f = pp.tile([128, 128], F32, tag="onesf", name="ones_f")
            # sel[:, t, :]: K=128 selection matrix broadcasting denominator
            # rows {64t, 64t+32} of dsb to output partitions [0:64], [64:128]
            sel = pp.tile([128, 2, 128], F32R, tag="sel", name="sel")
            sel_f = pp.tile([128, 2, 128], F32, tag="self", name="sel_f")

            # ---- input DMA: weights first (gate first proj groups), x spans
            # batched one-per-span and spread across engine queues ----
            # full x rows per k-tile: each partition line is one contiguous
            # 4KB DRAM row, spread across the three DMA-capable queues
            nc.sync.dma_start(out=wk_sb, in_=wk_d.ap().rearrange("(k p) n -> p k n", p=128))
            nc.scalar.dma_start(out=wq_sb, in_=wq_d.ap().rearrange("(k p) n -> p k n", p=128))
            nc.gpsimd.dma_start(out=wv_sb, in_=wv_d.ap().rearrange("(k p) n -> p k n", p=128))
            nc.scalar.dma_start(out=bqk_sb, in_=bqk_d[:, :])
            nc.scalar.dma_start(out=bv_sb, in_=bv_d[:, :])
            qs = [nc.sync, nc.scalar, nc.gpsimd]
            for k in range(NK):
                qs[k % 3].dma_start(out=xk[k], in_=xt_d[128 * k : 128 * (k + 1), :])
            nc.gpsimd.dma_start(out=wo_sb, in_=wo_d.ap().rearrange("(t p) n -> p t n", p=128))
            nc.vector.memset(ones_f, 1.0)
            nc.vector.tensor_copy(out=onesb, in_=ones_f)
            nc.vector.tensor_copy(out=ones_r, in_=ones_f[0:1, :])
            nc.vector.memset(sel_f, 0.0)
            for t in range(2):
                nc.vector.memset(sel_f[64 * t : 64 * t + 1, t, 0:64], 1.0)
                nc.vector.memset(sel_f[64 * t + 32 : 64 * t + 33, t, 64:128], 1.0)
            nc.vector.tensor_copy(out=sel, in_=sel_f)

            # ---- projection group emitters ----
            def kt_group(t, s, boot=False):
                if boot:
                    pb = ps.tile([128, 2, 512], F32, tag="s", bufs=2, name="pjb")[:, 0, :]
                else:
                    pb = ps.tile([128, 512], F32, tag="pj", name="pj")
                for k in range(NK):
                    nc.tensor.matmul(
                        pb[:, :],
                        wk_sb[:, k, 128 * t : 128 * (t + 1)],
                        xk[k][:, 512 * s : 512 * (s + 1)],
                        start=(k == 0),
                        stop=(k == NK - 1),
                    )
                nc.scalar.activation(
                    out=kt[t][:, 512 * s : 512 * (s + 1)],
                    in_=pb[:, :],
                    func=Identity,
                    bias=bqk_sb[:, 2 + t : 3 + t],
                )

            def qt_group(t, c):
                pb = ps.tile([128, 512], F32, tag="pj", name="pj")
                for k in range(NK):
                    nc.tensor.matmul(
                        pb[:, :],
                        wq_sb[:, k, 128 * t : 128 * (t + 1)],
                        xk[k][:, 512 * c : 512 * (c + 1)],
                        start=(k == 0),
                        stop=(k == NK - 1),
                    )
                nc.scalar.activation(
                    out=qt[t][:, 512 * c : 512 * (c + 1)],
                    in_=pb[:, :],
                    func=Identity,
                    bias=bqk_sb[:, t : t + 1],
                )

            def vt_group(ii):  # i-tiles 2*ii, 2*ii+1
                pb = ps.tile([128, 512], F32, tag="pj", name="pj")
                for j in range(2):
                    i = 2 * ii + j
                    for k in range(NK):
                        nc.tensor.matmul(
                            pb[:, 256 * j : 256 * (j + 1)],
                            xk[k][:, 128 * i : 128 * (i + 1)],
                            wv_sb[:, k, :],
                            start=(k == 0),
                            stop=False,
                        )
                    nc.tensor.matmul(
                        pb[:, 256 * j : 256 * (j + 1)],
                        ones_r,
                        bv_sb,
                        start=False,
                        stop=True,
                    )
                nc.scalar.activation(
                    out=vt[:, 2 * ii : 2 * ii + 2, :],
                    in_=pb.rearrange("p (j n) -> p j n", j=2),
                    func=Identity,
                )

            # fine-grained qt emitter: 2 matmuls per slot, psum bank held
            # across the slots of one group to avoid 8-MM pacer stalls
            qt_pb = {}

            def qt_part(t, c, k0, k1):
                key = (t, c)
                if key not in qt_pb:
                    qt_pb[key] = ps.tile([128, 512], F32, tag="pj", name="pj")
                pb = qt_pb[key]
                for k in range(k0, k1):
                    nc.tensor.matmul(
                        pb[:, :],
                        wq_sb[:, k, 128 * t : 128 * (t + 1)],
                        xk[k][:, 512 * c : 512 * (c + 1)],
                        start=(k == 0),
                        stop=(k == NK - 1),
                    )
                if k1 == NK:
                    nc.scalar.activation(
                        out=qt[t][:, 512 * c : 512 * (c + 1)],
                        in_=pb[:, :],
                        func=Identity,
                        bias=bqk_sb[:, t : t + 1],
                    )
                    del qt_pb[key]

            # just-in-time schedule: chunk-0 steps produce the remaining spans
            slots = {
                (0, 0): [lambda: vt_group(0)],
                (0, 1): [lambda: vt_group(1)],
                (0, 2): [lambda: kt_group(0, 1)],
                (0, 3): [lambda: kt_group(1, 1)],
                (0, 4): [lambda: vt_group(2)],
                (0, 5): [lambda: vt_group(3)],
                (0, 6): [lambda: kt_group(0, 2)],
                (0, 7): [lambda: kt_group(1, 2)],
                (0, 8): [lambda: vt_group(4)],
                (0, 9): [lambda: vt_group(5)],
                (0, 10): [lambda: kt_group(0, 3)],
                (0, 11): [lambda: kt_group(1, 3)],
                (0, 12): [lambda: vt_group(6)],
                (0, 13): [lambda: vt_group(7)],
                (0, 14): [lambda: qt_group(0, 1)],
                (0, 15): [lambda: qt_group(1, 1)],
            }
            for c in (1, 2):
                cn = c + 1
                for p in range(4):
                    slots[(c, 2 + p)] = [
                        lambda t=0, cn=cn, p=p: qt_part(t, cn, 2 * p, 2 * p + 2)
                    ]
                    slots[(c, 6 + p)] = [
                        lambda t=1, cn=cn, p=p: qt_part(t, cn, 2 * p, 2 * p + 2)
                    ]

            # bootstrap (overlapped with the x DMAs), pipelined through two
            # PSUM tags and ordered so step-0's t=0 can start after 2 groups
            kt_group(0, 0, boot=True)
            qt_group(0, 0)
            kt_group(1, 0, boot=True)
            qt_group(1, 0)

            # ---- bond DMA ring ----
            steps = [(c, i) for c in range(NCH) for i in range(LT)]
            bts = {}

            def bond_dma(n):
                if n >= len(steps):
                    return
                c, i = steps[n]
                bt = wk_pool.tile([128, 512], F16, tag="bt", bufs=6, name="bt")
                nc.sync.dma_start(
                    out=bt,
                    in_=bd_d[128 * i : 128 * (i + 1), 512 * c : 512 * (c + 1)],
                )
                bts[n] = bt

            for n in range(4):
                bond_dma(n)

            # ---- attention ----
            def pv_step(c, j, oacc, dn):
                first, last = (j == 0), (j == LT - 1)
                g = (j // 2) % 3
                for t in range(2):
                    idx = (j % 2) * 2 + t
                    for half in range(2):
                        h = 2 * t + half
                        nc.tensor.matmul(
                            oacc[t][64 * half : 64 * (half + 1), :],
                            vt[:, j, 64 * h : 64 * (h + 1)],
                            pt[g][:, idx, 512 * half : 512 * (half + 1)],
                            start=first,
                            stop=last,
                            tile_position=(0, 64 * half),
                        )
                for h in range(HPC):
                    t, half = h // 2, h % 2
                    idx = (j % 2) * 2 + t
                    nc.tensor.matmul(
                        dn[32 * h : 32 * h + 1, :],
                        onesb[:, 0:1],
                        pt[g][:, idx, 512 * half : 512 * (half + 1)],
                        start=first,
                        stop=last,
                        tile_position=(0, 32 * h),
                    )

            for c in range(NCH):
                oacc = [
                    ps.tile([128, 512], F32, tag=f"o{t}", name=f"oacc{t}")
                    for t in range(2)
                ]
                dn = ps.tile([128, 512], F32, tag="dn", name="dn")
                for i in range(LT):
                    n = c * LT + i
                    bond_dma(n + 4)
                    bt = bts.pop(n)
                    bt_b = bass.AP(
                        tensor=bt.tensor,
                        offset=bt.offset,
                        ap=[bt.ap[0], [0, 2]] + list(bt.ap[1:]),
                    )
                    g = (i // 2) % 3
                    for t in range(2):
                        sp = ps.tile([128, 2, 512], F32, tag="s", bufs=2, name="sp")
                        nc.tensor.matmul(
                            sp[:, 0, :],
                            kt[t][0:64, 128 * i : 128 * (i + 1)],
                            qt[t][0:64, 512 * c : 512 * (c + 1)],
                            start=True,
                            stop=True,
                        )
                        nc.tensor.matmul(
                            sp[:, 1, :],
                            kt[t][64:128, 128 * i : 128 * (i + 1)],
                            qt[t][64:128, 512 * c : 512 * (c + 1)],
                            start=True,
                            stop=True,
                            tile_position=(64, 0),
                        )
                        idx = (i % 2) * 2 + t
                        out_view = st[g][:, idx, :].rearrange("p (h q) -> p h q", h=2)
                        nc.vector.tensor_mul(out=out_view, in0=sp, in1=bt_b)
                    if i % 2 == 1:
                        with nc.allow_low_precision(reason="bf16 probs"):
                            nc.scalar.activation(out=pt[g], in_=st[g], func=Exp)
                    if i >= 4:
                        pv_step(c, i - 4, oacc, dn)
                    # slot copies go after exp so they never delay it on ACT
                    for fn in slots.get((c, i), ()):
                        fn()
                for j in range(LT - 4, LT):
                    pv_step(c, j, oacc, dn)

                # ---- chunk tail: denominators, normalize, out-proj ----
                dsb = wk_pool.tile([128, 512], F32R, tag="dsb", bufs=2, name="dsb")
                nc.scalar.activation(out=dsb, in_=dn, func=Identity)
                bcb = [
                    ps.tile([128, 512], F32, tag=("dn" if t == 0 else "pj"), name="bcb")
                    for t in range(2)
                ]
                for t in range(2):
                    nc.tensor.matmul(
                        bcb[t][:, :],
                        sel[:, t, :],
                        dsb[:, :],
                        start=True,
                        stop=True,
                    )
                rb = [
                    wk_pool.tile([128, 512], F32, tag="rb", bufs=2, name="rb")
                    for _ in range(2)
                ]
                on = wk_pool.tile([128, 2, 512], BF16, tag="on", bufs=2, name="on")
                with nc.allow_low_precision(reason="bf16 normalized O"):
                    for t in range(2):
                        nc.vector.reciprocal_approx_fast(out=rb[t], in_=bcb[t])
                        nc.vector.tensor_mul(out=on[:, t, :], in0=oacc[t], in1=rb[t])
                for jl in range(4):
                    j = 4 * c + jl
                    for dh in range(2):
                        yp = ps.tile(
                            [128, 512],
                            F32,
                            tag=("pj" if (2 * jl + dh) % 2 else "dn"),
                            name="yp",
                        )
                        for t in range(2):
                            nc.tensor.matmul(
                                yp[:, :],
                                on[:, t, 128 * jl : 128 * (jl + 1)],
                                wo_sb[:, t, 512 * dh : 512 * (dh + 1)],
                                start=(t == 0),
                                stop=(t == 1),
                            )
                        ys = wk_pool.tile([128, 512], BF16, tag="ys", bufs=4, name="ys")
                        with nc.allow_low_precision(reason="bf16 partial Y"):
                            nc.scalar.activation(out=ys, in_=yp, func=Identity)
                        nc.gpsimd.dma_start(
                            out=y_d[128 * j : 128 * (j + 1), 512 * dh : 512 * (dh + 1)],
                            in_=ys,
                        )

    nc.compile()
    return nc


def _get_nc():
    global _CACHED_NC
    if _CACHED_NC is None:
        _CACHED_NC = _build_nc()
    return _CACHED_NC


def _host_prep(x, bond_influence, Wq, bq, Wk, bk, Wv, bv, Wo):
    in_maps = []
    for core in range(N_CORES):
        b, g = core // HPC, core % HPC
        s = slice(g * DKG, (g + 1) * DKG)
        bq_g = (bq[s] / 8.0).astype(np.float32)
        bk_g = bk[s].astype(np.float32)
        bqk = np.stack(
            [bq_g[0:128], bq_g[128:256], bk_g[0:128], bk_g[128:256]], axis=1
        )
        in_maps.append(
            {
                "xt": np.ascontiguousarray(x[b].T).astype(bfloat16),
                "bd": np.ascontiguousarray(bond_influence[b].T.astype(np.float16)),
                "wq": np.ascontiguousarray(Wq[:, s] / 8.0).astype(bfloat16),
                "wk": np.ascontiguousarray(Wk[:, s]).astype(bfloat16),
                "wv": np.ascontiguousarray(Wv[:, s]).astype(bfloat16),
                "bqk": np.ascontiguousarray(bqk),
                "bv": np.ascontiguousarray(bv[s][None, :]).astype(bfloat16),
                "wo": np.ascontiguousarray(Wo[s, :]).astype(bfloat16),
            }
        )
    return in_maps


def kernel(
    x,
    bond_influence,
    Wq,
    bq,
    Wk,
    bk,
    Wv,
    bv,
    Wo,
    bo,
    _trace=False,
    _trace_out=None,
):
    x = np.asarray(x, dtype=np.float32)
    bond_influence = np.asarray(bond_influence, dtype=np.float32)
    args = [np.asarray(a, dtype=np.float32) for a in (Wq, bq, Wk, bk, Wv, bv, Wo)]
    bo = np.asarray(bo, dtype=np.float32)

    nc = _get_nc()
    in_maps = _host_prep(x, bond_influence, *args)
    kwargs = {}
    if _trace:
        kwargs = dict(trace=True, tmpdir=_trace_out)
    res = run_bass_kernel_spmd(nc, in_maps, list(range(N_CORES)), **kwargs)

    out = np.zeros((B, L, D), dtype=np.float32)
    for b in range(B):
        acc = res.results[4 * b]["y"].astype(np.float32)
        for g in range(1, HPC):
            acc = acc + res.results[4 * b + g]["y"].astype(np.float32)
        out[b] = acc + bo[None, :]
    if _trace:
        return out, res
    return out


# revision 35
# speedup vs baseline: 1.3486x; 1.0261x over previous
"""BondInfluenceSelfAttention TRN2 kernel (v2).

Full-input contract: kernel(**inputs) takes the complete unsharded inputs and
returns the full [B, L, D] output. Internally shards across 8 NeuronCores:
core c handles batch b = c // 4 and head-group g = c % 4 (4 heads, 256 dk
dims). Each core computes its heads' attention plus the partial output
projection through its 256 rows of Wo; the host sums the 4 partials per batch
and adds bo.

v2 design (vs the v1 in git history):
- bf16 operands for every matmul (proj, scores, PV, out-proj); fp32 PSUM.
- Score matmuls (K=64) packed two-per-array via tile_position row groups;
  PV matmuls (M=64) packed two-per-array via column groups.
- Softmax denominators from 4-way column-packed ones^T @ P matmuls into one
  PSUM bank; normalization deferred to the chunk tail (reciprocal_approx_fast
  on DVE + K=1 broadcast matmuls), killing v1's 53us of DVE RECIPROCAL.
- ACT does only: exp in [128,4096] ops, Q/K/V PSUM->SBUF copies (with bias),
  Y copies. DVE does only: bond-multiply (PSUM->SBUF), recip, normalize.
- QKV projection matmul groups are interleaved into chunk 0's attention steps
  (just-in-time spans) so DVE/ACT never sit idle behind a serial proj phase.
"""

import numpy as np

try:
    import concourse.bass as bass  # noqa: F401
except ImportError:  # pragma: no cover
    import sys

    sys.path.insert(0, "/opt/trn_rl_repo")
    import concourse.bass as bass  # noqa: F401

import concourse.bacc as bacc
import concourse.mybir as mybir
import concourse.tile as tile
from concourse.bass_utils import run_bass_kernel_spmd
from ml_dtypes import bfloat16

F32 = mybir.dt.float32
F32R = mybir.dt.float32r
BF16 = mybir.dt.bfloat16
F16 = mybir.dt.float16

D = 1024  # d_model
L = 2048  # sequence length
B = 2  # batch
HPC = 4  # heads per core
DKG = 256  # dk dims per core (4 heads x 64)
NK = D // 128  # 8 contraction k-tiles for the projections
LT = L // 128  # 16 L k-position tiles
NCH = L // 512  # 4 L_q chunks
NSP = 4  # 512-wide k-position spans (kt/vt production granularity)
N_CORES = 8

_CACHED_NC = None


def _build_nc():
    nc = bacc.Bacc("TRN2", target_bir_lowering=False, debug=False, num_devices=N_CORES)

    # weights arrive host-pre-transposed to partition-major [128, k, n] so
    # each DMA partition line is one contiguous DRAM run
    xt_d = nc.declare_dram_parameter("xt", [D, L], BF16, isOutput=False)
    bd_d = nc.declare_dram_parameter("bd", [L, L], F16, isOutput=False)
    wq_d = nc.declare_dram_parameter("wq", [128, NK * DKG], BF16, isOutput=False)
    wk_d = nc.declare_dram_parameter("wk", [128, NK * DKG], BF16, isOutput=False)
    wv_d = nc.declare_dram_parameter("wv", [128, NK * DKG], BF16, isOutput=False)
    bqk_d = nc.declare_dram_parameter("bqk", [128, 4], F32, isOutput=False)
    bv_d = nc.declare_dram_parameter("bv", [1, DKG], BF16, isOutput=False)
    wo_d = nc.declare_dram_parameter("wo", [128, 2 * D], BF16, isOutput=False)
    y_d = nc.declare_dram_parameter("y", [L, D], BF16, isOutput=True)

    Exp = mybir.ActivationFunctionType.Exp
    Identity = mybir.ActivationFunctionType.Identity

    with tile.TileContext(nc) as tc:
        with tc.tile_pool(name="persist", bufs=1) as pp, tc.tile_pool(
            name="work", bufs=1
        ) as wk_pool, tc.tile_pool(name="ps", bufs=1, space="PSUM") as ps:
            # ---- persistent SBUF ----
            xk = [
                pp.tile([128, L], BF16, tag=f"xk{k}", name=f"xk{k}")
                for k in range(NK)
            ]
            wq_sb = pp.tile([128, NK, DKG], BF16, tag="wq", name="wq_sb")
            wk_sb = pp.tile([128, NK, DKG], BF16, tag="wk", name="wk_sb")
            wv_sb = pp.tile([128, NK, DKG], BF16, tag="wv", name="wv_sb")
            qt = [pp.tile([128, L], BF16, tag=f"qt{t}", name=f"qt{t}") for t in range(2)]
            kt = [pp.tile([128, L], BF16, tag=f"kt{t}", name=f"kt{t}") for t in range(2)]
            vt = pp.tile([128, LT, DKG], BF16, tag="vt", name="vt")
            st = [
                pp.tile([128, 4, 1024], F32, tag=f"st{g}", name=f"st{g}")
                for g in range(3)
            ]
            pt = [
                pp.tile([128, 4, 1024], BF16, tag=f"pt{g}", name=f"pt{g}")
                for g in range(3)
            ]
            wo_sb = pp.tile([128, 2, D], BF16, tag="wo", name="wo_sb")
            bqk_sb = pp.tile([128, 4], F32, tag="bqk", name="bqk_sb")
            bv_sb = pp.tile([1, DKG], BF16, tag="bv", name="bv_sb")
            ones_r = pp.tile([1, 128], BF16, tag="onesr", name="ones_r")
            onesb = pp.tile([128, 128], BF16, tag="onesb", name="onesb")
            ones_f = pp.tile([128, 128], F32, tag="onesf", name="ones_f")
            # sel[:, t, :]: K=128 selection matrix broadcasting denominator
            # rows {64t, 64t+32} of dsb to output partitions [0:64], [64:128]
            sel = pp.tile([128, 2, 128], F32R, tag="sel", name="sel")
            sel_f = pp.tile([128, 2, 128], F32, tag="self", name="sel_f")

            # ---- input DMA: weights first (gate first proj groups), x spans
            # batched one-per-span and spread across engine queues ----
            # weights: contiguous partition-major loads; x: span-0 columns of
            # every k-tile first (gates the bootstrap groups), rest second
            nc.sync.dma_start(out=wk_sb, in_=wk_d.ap().rearrange("p (k n) -> p k n", k=NK))
            nc.scalar.dma_start(out=wq_sb, in_=wq_d.ap().rearrange("p (k n) -> p k n", k=NK))
            nc.gpsimd.dma_start(out=wv_sb, in_=wv_d.ap().rearrange("p (k n) -> p k n", k=NK))
            nc.scalar.dma_start(out=bqk_sb, in_=bqk_d[:, :])
            nc.scalar.dma_start(out=bv_sb, in_=bv_d[:, :])
            qs = [nc.sync, nc.scalar, nc.gpsimd]
            for k in range(NK):
                qs[k % 3].dma_start(
                    out=xk[k][:, 0:512], in_=xt_d[128 * k : 128 * (k + 1), 0:512]
                )
            for k in range(NK):
                qs[k % 3].dma_start(
                    out=xk[k][:, 512:L], in_=xt_d[128 * k : 128 * (k + 1), 512:L]
                )
            nc.gpsimd.dma_start(out=wo_sb, in_=wo_d.ap().rearrange("p (t n) -> p t n", t=2))
            nc.vector.memset(ones_f, 1.0)
            nc.vector.tensor_copy(out=onesb, in_=ones_f)
            nc.vector.tensor_copy(out=ones_r, in_=ones_f[0:1, :])
            nc.vector.memset(sel_f, 0.0)
            for t in range(2):
                nc.vector.memset(sel_f[64 * t : 64 * t + 1, t, 0:64], 1.0)
                nc.vector.memset(sel_f[64 * t + 32 : 64 * t + 33, t, 64:128], 1.0)
            nc.vector.tensor_copy(out=sel, in_=sel_f)

            # ---- projection group emitters ----
            def kt_group(t, s, boot=False):
                if boot:
                    pb = ps.tile([128, 2, 512], F32, tag="s", bufs=2, name="pjb")[:, 0, :]
                else:
                    pb = ps.tile([128, 512], F32, tag="pj", name="pj")
                for k in range(NK):
                    nc.tensor.matmul(
                        pb[:, :],
                        wk_sb[:, k, 128 * t : 128 * (t + 1)],
                        xk[k][:, 512 * s : 512 * (s + 1)],
                        start=(k == 0),
                        stop=(k == NK - 1),
                    )
                nc.scalar.activation(
                    out=kt[t][:, 512 * s : 512 * (s + 1)],
                    in_=pb[:, :],
                    func=Identity,
                    bias=bqk_sb[:, 2 + t : 3 + t],
                )

            def qt_group(t, c):
                pb = ps.tile([128, 512], F32, tag="pj", name="pj")
                for k in range(NK):
                    nc.tensor.matmul(
                        pb[:, :],
                        wq_sb[:, k, 128 * t : 128 * (t + 1)],
                        xk[k][:, 512 * c : 512 * (c + 1)],
                        start=(k == 0),
                        stop=(k == NK - 1),
                    )
                nc.scalar.activation(
                    out=qt[t][:, 512 * c : 512 * (c + 1)],
                    in_=pb[:, :],
                    func=Identity,
                    bias=bqk_sb[:, t : t + 1],
                )

            def vt_group(ii):  # i-tiles 2*ii, 2*ii+1
                pb = ps.tile([128, 512], F32, tag="pj", name="pj")
                for j in range(2):
                    i = 2 * ii + j
                    for k in range(NK):
                        nc.tensor.matmul(
                            pb[:, 256 * j : 256 * (j + 1)],
                            xk[k][:, 128 * i : 128 * (i + 1)],
                            wv_sb[:, k, :],
                            start=(k == 0),
                            stop=False,
                        )
                    nc.tensor.matmul(
                        pb[:, 256 * j : 256 * (j + 1)],
                        ones_r,
                        bv_sb,
                        start=False,
                        stop=True,
                    )
                nc.scalar.activation(
                    out=vt[:, 2 * ii : 2 * ii + 2, :],
                    in_=pb.rearrange("p (j n) -> p j n", j=2),
                    func=Identity,
                )

            # fine-grained qt emitter: 2 matmuls per slot, psum bank held
            # across the slots of one group to avoid 8-MM pacer stalls
            qt_pb = {}

            def qt_part(t, c, k0, k1):
                key = (t, c)
                if key not in qt_pb:
                    qt_pb[key] = ps.tile([128, 512], F32, tag="pj", name="pj")
                pb = qt_pb[key]
                for k in range(k0, k1):
                    nc.tensor.matmul(
                        pb[:, :],
                        wq_sb[:, k, 128 * t : 128 * (t + 1)],
                        xk[k][:, 512 * c : 512 * (c + 1)],
                        start=(k == 0),
                        stop=(k == NK - 1),
                    )
                if k1 == NK:
                    nc.scalar.activation(
                        out=qt[t][:, 512 * c : 512 * (c + 1)],
                        in_=pb[:, :],
                        func=Identity,
                        bias=bqk_sb[:, t : t + 1],
                    )
                    del qt_pb[key]

            # just-in-time schedule: chunk-0 steps produce the remaining spans
            slots = {
                (0, 0): [lambda: vt_group(0)],
                (0, 1): [lambda: vt_group(1)],
                (0, 2): [lambda: kt_group(0, 1)],
                (0, 3): [lambda: kt_group(1, 1)],
                (0, 4): [lambda: vt_group(2)],
                (0, 5): [lambda: vt_group(3)],
                (0, 6): [lambda: kt_group(0, 2)],
                (0, 7): [lambda: kt_group(1, 2)],
                (0, 8): [lambda: vt_group(4)],
                (0, 9): [lambda: vt_group(5)],
                (0, 10): [lambda: kt_group(0, 3)],
                (0, 11): [lambda: kt_group(1, 3)],
                (0, 12): [lambda: vt_group(6)],
                (0, 13): [lambda: vt_group(7)],
                (0, 14): [lambda: qt_group(0, 1)],
                (0, 15): [lambda: qt_group(1, 1)],
            }
            for c in (1, 2):
                cn = c + 1
                for p in range(4):
                    slots[(c, 2 + p)] = [
                        lambda t=0, cn=cn, p=p: qt_part(t, cn, 2 * p, 2 * p + 2)
                    ]
                    slots[(c, 6 + p)] = [
                        lambda t=1, cn=cn, p=p: qt_part(t, cn, 2 * p, 2 * p + 2)
                    ]

            # bootstrap (overlapped with the x DMAs), pipelined through two
            # PSUM tags and ordered so step-0's t=0 can start after 2 groups
            kt_group(0, 0, boot=True)
            qt_group(0, 0)
            kt_group(1, 0, boot=True)
            qt_group(1, 0)

            # ---- bond DMA ring ----
            steps = [(c, i) for c in range(NCH) for i in range(LT)]
            bts = {}

            def bond_dma(n):
                if n >= len(steps):
                    return
                c, i = steps[n]
                bt = wk_pool.tile([128, 512], F16, tag="bt", bufs=6, name="bt")
                nc.sync.dma_start(
                    out=bt,
                    in_=bd_d[128 * i : 128 * (i + 1), 512 * c : 512 * (c + 1)],
                )
                bts[n] = bt

            for n in range(4):
                bond_dma(n)

            # ---- attention ----
            def pv_step(c, j, oacc, dn):
                first, last = (j == 0), (j == LT - 1)
                g = (j // 2) % 3
                for t in range(2):
                    idx = (j % 2) * 2 + t
                    for half in range(2):
                        h = 2 * t + half
                        nc.tensor.matmul(
                            oacc[t][64 * half : 64 * (half + 1), :],
                            vt[:, j, 64 * h : 64 * (h + 1)],
                            pt[g][:, idx, 512 * half : 512 * (half + 1)],
                            start=first,
                            stop=last,
                            tile_position=(0, 64 * half),
                        )
                for h in range(HPC):
                    t, half = h // 2, h % 2
                    idx = (j % 2) * 2 + t
                    nc.tensor.matmul(
                        dn[32 * h : 32 * h + 1, :],
                        onesb[:, 0:1],
                        pt[g][:, idx, 512 * half : 512 * (half + 1)],
                        start=first,
                        stop=last,
                        tile_position=(0, 32 * h),
                    )

            for c in range(NCH):
                oacc = [
                    ps.tile([128, 512], F32, tag=f"o{t}", name=f"oacc{t}")
                    for t in range(2)
                ]
                dn = ps.tile([128, 512], F32, tag="dn", name="dn")
                for i in range(LT):
                    n = c * LT + i
                    bond_dma(n + 4)
                    bt = bts.pop(n)
                    bt_b = bass.AP(
                        tensor=bt.tensor,
                        offset=bt.offset,
                        ap=[bt.ap[0], [0, 2]] + list(bt.ap[1:]),
                    )
                    g = (i // 2) % 3
                    for t in range(2):
                        sp = ps.tile([128, 2, 512], F32, tag="s", bufs=2, name="sp")
                        nc.tensor.matmul(
                            sp[:, 0, :],
                            kt[t][0:64, 128 * i : 128 * (i + 1)],
                            qt[t][0:64, 512 * c : 512 * (c + 1)],
                            start=True,
                            stop=True,
                        )
                        nc.tensor.matmul(
                            sp[:, 1, :],
                            kt[t][64:128, 128 * i : 128 * (i + 1)],
                            qt[t][64:128, 512 * c : 512 * (c + 1)],
                            start=True,
                            stop=True,
                            tile_position=(64, 0),
                        )
                        idx = (i % 2) * 2 + t
                        out_view = st[g][:, idx, :].rearrange("p (h q) -> p h q", h=2)
                        nc.vector.tensor_mul(out=out_view, in0=sp, in1=bt_b)
                    if i % 2 == 1:
                        with nc.allow_low_precision(reason="bf16 probs"):
                            nc.scalar.activation(out=pt[g], in_=st[g], func=Exp)
                    if i >= 4:
                        pv_step(c, i - 4, oacc, dn)
                    # slot copies go after exp so they never delay it on ACT
                    for fn in slots.get((c, i), ()):
                        fn()
                for j in range(LT - 4, LT):
                    pv_step(c, j, oacc, dn)

                # ---- chunk tail: denominators, normalize, out-proj ----
                dsb = wk_pool.tile([128, 512], F32R, tag="dsb", bufs=2, name="dsb")
                nc.scalar.activation(out=dsb, in_=dn, func=Identity)
                bcb = [
                    ps.tile([128, 512], F32, tag=("dn" if t == 0 else "pj"), name="bcb")
                    for t in range(2)
                ]
                for t in range(2):
                    nc.tensor.matmul(
                        bcb[t][:, :],
                        sel[:, t, :],
                        dsb[:, :],
                        start=True,
                        stop=True,
                    )
                rb = [
                    wk_pool.tile([128, 512], F32, tag="rb", bufs=2, name="rb")
                    for _ in range(2)
                ]
                on = wk_pool.tile([128, 2, 512], BF16, tag="on", bufs=2, name="on")
                with nc.allow_low_precision(reason="bf16 normalized O"):
                    for t in range(2):
                        nc.vector.reciprocal_approx_fast(out=rb[t], in_=bcb[t])
                        nc.vector.tensor_mul(out=on[:, t, :], in0=oacc[t], in1=rb[t])
                for jl in range(4):
                    j = 4 * c + jl
                    for dh in range(2):
                        yp = ps.tile(
                            [128, 512],
                            F32,
                            tag=("pj" if (2 * jl + dh) % 2 else "dn"),
                            name="yp",
                        )
                        for t in range(2):
                            nc.tensor.matmul(
                                yp[:, :],
                                on[:, t, 128 * jl : 128 * (jl + 1)],
                                wo_sb[:, t, 512 * dh : 512 * (dh + 1)],
                                start=(t == 0),
                                stop=(t == 1),
                            )
                        ys = wk_pool.tile([128, 512], BF16, tag="ys", bufs=4, name="ys")
                        with nc.allow_low_precision(reason="bf16 partial Y"):
                            nc.scalar.activation(out=ys, in_=yp, func=Identity)
                        nc.gpsimd.dma_start(
                            out=y_d[128 * j : 128 * (j + 1), 512 * dh : 512 * (dh + 1)],
                            in_=ys,
                        )

    nc.compile()
    return nc


def _get_nc():
    global _CACHED_NC
    if _CACHED_NC is None:
        _CACHED_NC = _build_nc()
    return _CACHED_NC


def _host_prep(x, bond_influence, Wq, bq, Wk, bk, Wv, bv, Wo):
    in_maps = []
    for core in range(N_CORES):
        b, g = core // HPC, core % HPC
        s = slice(g * DKG, (g + 1) * DKG)
        bq_g = (bq[s] / 8.0).astype(np.float32)
        bk_g = bk[s].astype(np.float32)
        bqk = np.stack(
            [bq_g[0:128], bq_g[128:256], bk_g[0:128], bk_g[128:256]], axis=1
        )
        in_maps.append(
            {
                "xt": np.ascontiguousarray(x[b].T).astype(bfloat16),
                "bd": np.ascontiguousarray(bond_influence[b].T.astype(np.float16)),
                "wq": np.ascontiguousarray(
                    (Wq[:, s] / 8.0).reshape(NK, 128, DKG).transpose(1, 0, 2).reshape(128, -1)
                ).astype(bfloat16),
                "wk": np.ascontiguousarray(
                    Wk[:, s].reshape(NK, 128, DKG).transpose(1, 0, 2).reshape(128, -1)
                ).astype(bfloat16),
                "wv": np.ascontiguousarray(
                    Wv[:, s].reshape(NK, 128, DKG).transpose(1, 0, 2).reshape(128, -1)
                ).astype(bfloat16),
                "bqk": np.ascontiguousarray(bqk),
                "bv": np.ascontiguousarray(bv[s][None, :]).astype(bfloat16),
                "wo": np.ascontiguousarray(
                    Wo[s, :].reshape(2, 128, D).transpose(1, 0, 2).reshape(128, -1)
                ).astype(bfloat16),
            }
        )
    return in_maps


def kernel(
    x,
    bond_influence,
    Wq,
    bq,
    Wk,
    bk,
    Wv,
    bv,
    Wo,
    bo,
    _trace=False,
    _trace_out=None,
):
    x = np.asarray(x, dtype=np.float32)
    bond_influence = np.asarray(bond_influence, dtype=np.float32)
    args = [np.asarray(a, dtype=np.float32) for a in (Wq, bq, Wk, bk, Wv, bv, Wo)]
    bo = np.asarray(bo, dtype=np.float32)

    nc = _get_nc()
    in_maps = _host_prep(x, bond_influence, *args)
    kwargs = {}
    if _trace:
        kwargs = dict(trace=True, tmpdir=_trace_out)
    res = run_bass_kernel_spmd(nc, in_maps, list(range(N_CORES)), **kwargs)

    out = np.zeros((B, L, D), dtype=np.float32)
    for b in range(B):
        acc = res.results[4 * b]["y"].astype(np.float32)
        for g in range(1, HPC):
            acc = acc + res.results[4 * b + g]["y"].astype(np.float32)
        out[b] = acc + bo[None, :]
    if _trace:
        return out, res
    return out
